# revision 15
# baseline (speedup 1.0000x reference)
"""Trainium2 Bass kernel for nn_InterViews (retrieval_knn).

Computes, per batch item b: the variance (ddof=1) of the strict-upper-
triangular entries of the cosine-similarity Gram matrix between the
item's V=16 views, negated.

Strategy (data-parallel over bs across 8 cores, 128 items/core):
  - Host: shard rows so core k holds 16 groups of 8 items (each group =
    128 rows = 8 items x 16 views), cast to fp8 e4m3 (TRN variant,
    max 240; ~7e-3 end-to-end rel err vs fp32, PSUM accumulation stays
    fp32), and pre-transpose to xh[c, g, j, v] = x[g*128+v, j*128+c].
    Each (c, g) row is 4096 contiguous bytes, so the device DMA is pure
    streaming (no transpose descriptors).
  - Device: one merged-const DMA + 16 single-group DMAs all issued
    up-front on the sync HWDGE queue; all 16 group tiles stay
    SBUF-resident (64 KB/partition) so nothing is gated on buffer reuse
    and the 16 DMA engines stream the 8 MB input back-to-back
    (~22.5 us).  While the first group is in flight, dummy fp8 matmuls
    keep the PE busy so its DVFS ramp (0.65 -> 1.2 -> 2.4 GHz over 3 us
    of continuous execution) completes before the real Grams start.
  - Per group: 16 fp8 DoubleRow matmuls (two 128-channel k-tiles per
    instruction) accumulating G = A A^T in fp32 PSUM (~78 ns/matmul at
    full clock; 4 quads of 4 groups ride a 4-deep PSUM ring).
  - Postprocessing per quad, phase-split so every consumer is issued one
    quad after its producer and no engine blocks the PE queue head:
      A (after grams q):  n2 = diag(G) via mask+rowsum (DVE); rec =
          1/n2, inv = sqrt(rec); xd4 = I*inv
      B (after grams q+1): invT = BDO^T @ xd (PE, fp32r), tmp = G*invT,
          t1 = rowsum, wst = tmp^2, r2 = rowsum, stats = (t1*inv, r2*rec)
      C (after grams q+2): [s1,s2] = BD^T @ stats (PE, fp32r), then
          out = ((s1)*S1SCL)^2 - s2/238 folded into two activations.
    The last quad's phase A is half-interleaved into its gram stream to
    shorten the end-of-kernel chain.
"""

import numpy as np

try:
    import concourse.bass as bass  # noqa: F401
except ImportError:  # container installs the repo at /opt/trn_rl_repo
    import sys

    sys.path.insert(0, "/opt/trn_rl_repo")

import ml_dtypes

import concourse.bass as bass
import concourse.mybir as mybir
import concourse.tile as tile
from concourse import bacc
from concourse.bass_utils import run_bass_kernel_spmd

F32 = mybir.dt.float32
F32R = mybir.dt.float32r
F8 = mybir.dt.float8e4
NP_F8 = ml_dtypes.float8_e4m3
P = 128          # partitions / rows per group
C = 4096         # channels
V = 16           # views per item
NCORES = 8
BS = 1024        # total batch
BS_CORE = BS // NCORES   # 128 items per core
IPG = P // V             # 8 items per group
NG = BS_CORE // IPG      # 16 groups per core
NCH = C // P             # 32 channel chunks
QG = 4                   # groups per postprocessing quad
NQ = NG // QG

MULT = mybir.AluOpType.mult
ADD = mybir.AluOpType.add
AF = mybir.ActivationFunctionType
AXX = mybir.AxisListType.X
DR = mybir.MatmulPerfMode.DoubleRow

S1SCL = 1.0 / np.sqrt(240.0 * 238.0)

# PE warmup: dummy matmuls that bridge the DMA latency window and finish
# the tensor engine's DVFS ramp before the first real Gram.
NWARM_BIG = 8     # [128, 512] fp8 matmuls (~0.43 us each at mid clock)
NWARM_SMALL = 8   # [128, 128] fp8 matmuls (fine-grained end of the bridge)


def _pe_dep_join(nc, jscr, t32a, t32b):
    """Tiny PE matmul reading a 32x32 corner of a freshly DMA'd tile,
    absorbing its DMA semaphore wait into PE's observed clock so the
    following real Matmult instructions need at most one sync wait each
    (TRN2 HW limit on Matmult)."""
    nc.tensor.matmul(jscr, t32a, t32b, skip_group_check=True)


def build_tile_kernel(tc, outs, ins):
    """ins = [xh [P, NG, NCH, P] f8e4, cst [P, 3P] f32 = [I | BD-I | BD]]
    outs = [y [IPG, NG] f32]  (y[b, g] = result for local item g*8+b)
    """
    nc = tc.nc
    xh, cst = ins
    (y,) = outs

    from contextlib import ExitStack

    with ExitStack() as ctx:
        x_pool = ctx.enter_context(tc.tile_pool(name="x", bufs=NG))
        g_psum = ctx.enter_context(tc.tile_pool(name="gp", bufs=NQ, space="PSUM"))
        pp_psum = ctx.enter_context(tc.tile_pool(name="pp", bufs=2, space="PSUM"))
        j_psum = ctx.enter_context(tc.tile_pool(name="jp", bufs=1, space="PSUM"))
        mid_pool = ctx.enter_context(tc.tile_pool(name="mid", bufs=2))
        sm_pool = ctx.enter_context(tc.tile_pool(name="sm", bufs=2))
        c_pool = ctx.enter_context(tc.tile_pool(name="const", bufs=1))

        jscr = j_psum.tile([32, 32], F32)

        # Warmup scratch: memset'd fp8 operand; output borrows the first
        # slot of the gram PSUM ring (quad 3 reuses it later — a
        # same-engine in-order reuse, so no extra semaphore wait).
        wsrc = c_pool.tile([P, 4, P], F8)
        nc.vector.memset(wsrc[:], 0.0)
        wdst = g_psum.tile([P, QG * P], F32, name="gps", tag="gps")

        xtiles = [None] * NG

        # Consts first (one small DMA, ~0.2 us of stream), then all 16
        # group DMAs, all on the sync queue so completions arrive in
        # consumption order.
        cstt = c_pool.tile([P, 3 * P], F32)
        nc.sync.dma_start(cstt[:], cst[:, :])
        ident = cstt[:, 0:P]
        # BD masks feed fp32r matmuls, whose inputs must come from a
        # rounding-capable producer — rewrite them through an Act copy.
        bdrr = c_pool.tile([P, 2 * P], F32R)
        nc.scalar.copy(bdrr[:], cstt[:, P:3 * P])
        bdot = bdrr[:, 0:P]
        bdt = bdrr[:, P:2 * P]

        for g in range(NG):
            xg = x_pool.tile([P, NCH, P], F8, tag="x")
            nc.sync.dma_start(xg[:, :, :], xh[:, g, :, :])
            xtiles[g] = xg

        # PE warmup: runs while the first group is still in flight.
        for i in range(NWARM_BIG):
            nc.tensor.matmul(wdst[:], wsrc[:, 0, :], wsrc[:],
                             skip_group_check=True)
        for i in range(NWARM_SMALL):
            nc.tensor.matmul(wdst[:, 0:P], wsrc[:, 0, :], wsrc[:, 0, :],
                             skip_group_check=True)

        _pe_dep_join(nc, jscr[:], bdot[0:32, 0:32], bdot[0:32, 0:32])

        stage = c_pool.tile([P, NG], F32)
        identb = ident.unsqueeze(1).broadcast_to([P, QG, P])

        gtiles = [None] * NQ
        # cross-phase postproc state per quad
        pps = [dict() for _ in range(NQ)]

        def grams(q, last=False):
            """Gram matmuls for the 4 groups of quad q.  For the last
            quad, half of phase_a's DVE work and an Act copy of G to
            SBUF are interleaved mid-quad (DVE/Act only, so the PE
            stream is untouched) to shorten the end-of-kernel chain."""
            gps = g_psum.tile([P, QG * P], F32, name="gps", tag="gps")
            gtiles[q] = gps
            for gl in range(QG):
                g = q * QG + gl
                xg = xtiles[g]
                for j in range(NCH // 2):
                    nc.tensor.matmul(
                        gps[:, gl * P:(gl + 1) * P],
                        xg[:, 2 * j:2 * j + 2, :],
                        xg[:, 2 * j:2 * j + 2, :],
                        start=(j == 0),
                        stop=(j == NCH // 2 - 1),
                        perf_mode=DR,
                        skip_group_check=True,
                    )
                if last and gl == 1:
                    st = pps[q]
                    hb = slice(0, 2 * P)
                    ident2 = ident.unsqueeze(1).broadcast_to([P, 2, P])
                    gs4 = mid_pool.tile([P, QG * P], F32, tag="gs")
                    nc.scalar.copy(gs4[:, hb], gps[:, hb])
                    scr4 = mid_pool.tile([P, QG * P], F32, tag="scr")
                    nc.vector.tensor_mul(
                        scr4[:, hb].rearrange("p (i q) -> p i q", i=2),
                        gps[:, hb].rearrange("p (i q) -> p i q", i=2), ident2,
                    )
                    n2q = sm_pool.tile([P, QG], F32, tag="n2")
                    nc.vector.reduce_sum(
                        n2q[:, 0:2],
                        scr4[:, hb].rearrange("p (i q) -> p i q", i=2),
                        axis=AXX,
                    )
                    st["gs4"], st["scr4"], st["n2q"] = gs4, scr4, n2q

        def phase_a(q):
            """diag -> rec/inv -> xd4 = I*inv (no PE)."""
            gps = gtiles[q]
            st = pps[q]
            if "n2q" in st:  # last quad: first half already done mid-grams
                hb = slice(2 * P, QG * P)
                ident2 = ident.unsqueeze(1).broadcast_to([P, 2, P])
                scr4, n2q = st["scr4"], st["n2q"]
                nc.scalar.copy(st["gs4"][:, hb], gps[:, hb])
                nc.vector.tensor_mul(
                    scr4[:, hb].rearrange("p (i q) -> p i q", i=2),
                    gps[:, hb].rearrange("p (i q) -> p i q", i=2), ident2,
                )
                nc.vector.reduce_sum(
                    n2q[:, 2:QG],
                    scr4[:, hb].rearrange("p (i q) -> p i q", i=2), axis=AXX,
                )
            else:
                scr4 = mid_pool.tile([P, QG * P], F32, tag="scr")
                nc.vector.tensor_mul(
                    scr4[:].rearrange("p (i q) -> p i q", i=QG),
                    gps[:].rearrange("p (i q) -> p i q", i=QG), identb,
                )
                n2q = sm_pool.tile([P, QG], F32, tag="n2")
                nc.vector.reduce_sum(
                    n2q[:], scr4[:].rearrange("p (i q) -> p i q", i=QG), axis=AXX
                )
            recq = sm_pool.tile([P, QG], F32, tag="rec")
            nc.vector.reciprocal(recq[:], n2q[:])
            invq = sm_pool.tile([P, QG], F32, tag="inv")
            nc.scalar.activation(invq[:], recq[:], AF.Sqrt)
            invb = invq[:].unsqueeze(2).broadcast_to([P, QG, P])
            xd4 = mid_pool.tile([P, QG * P], F32R, tag="xd")
            nc.vector.tensor_mul(
                xd4[:].rearrange("p (i q) -> p i q", i=QG), identb, invb
            )
            st["recq"], st["invq"], st["xd4"] = recq, invq, xd4

        def phase_b(q):
            """invT = BDO^T@xd (PE fp32r, xd ready a quad ago) -> stats."""
            gps = gtiles[q]
            st = pps[q]
            ips4 = pp_psum.tile([P, QG * P], F32, tag="ips")
            nc.tensor.matmul(ips4[:], bdot.bitcast(F32R),
                             st["xd4"][:], skip_group_check=True)
            tmp4 = mid_pool.tile([P, QG * P], F32, tag="tmp")
            if "gs4" in st:  # last quad: G is in SBUF, read invT from PSUM
                nc.vector.tensor_mul(tmp4[:], st["gs4"][:], ips4[:])
            else:
                invT4 = mid_pool.tile([P, QG * P], F32, tag="invT")
                nc.scalar.copy(invT4[:], ips4[:])
                nc.vector.tensor_mul(tmp4[:], gps[:], invT4[:])
            t1q = sm_pool.tile([P, QG], F32, tag="t1")
            nc.vector.reduce_sum(
                t1q[:], tmp4[:].rearrange("p (i q) -> p i q", i=QG), axis=AXX
            )
            wst4 = mid_pool.tile([P, QG * P], F32, tag="wst")
            nc.scalar.activation(wst4[:], tmp4[:], AF.Square)
            r2q = sm_pool.tile([P, QG], F32, tag="r2")
            nc.vector.reduce_sum(
                r2q[:], wst4[:].rearrange("p (i q) -> p i q", i=QG), axis=AXX
            )
            # s1c = t1*inv ; s2c = r2*rec (rec = inv^2), interleaved
            stats = sm_pool.tile([P, 2 * QG], F32R, tag="stats")
            nc.vector.tensor_mul(stats[:, 0:2 * QG:2], t1q[:], st["invq"][:])
            nc.vector.tensor_mul(stats[:, 1:2 * QG:2], r2q[:], st["recq"][:])
            st["stats"] = stats

        def phase_c(q):
            """[s1,s2] = BD^T@stats (PE fp32r, stats ready a quad ago)."""
            st = pps[q]
            sps = j_psum.tile([P, 2 * QG], F32, tag="sps")
            nc.tensor.matmul(sps[:], bdt.bitcast(F32R),
                             st["stats"][:], skip_group_check=True)
            # out = (s1*S1SCL)^2 - s2/238  (= -var)
            qv = sm_pool.tile([P, QG], F32, tag="qv")
            nc.scalar.activation(qv[:], sps[:, 0:2 * QG:2], AF.Square, scale=S1SCL)
            wv = sm_pool.tile([P, QG], F32, tag="wv")
            nc.scalar.mul(wv[:], sps[:, 1:2 * QG:2], -1.0 / 238.0)
            nc.vector.tensor_add(stage[:, q * QG:(q + 1) * QG], qv[:], wv[:])
            src = stage[:].rearrange("(b r) g -> b r g", r=V)[:, 0, q * QG:(q + 1) * QG]
            nc.sync.dma_start(y[:, q * QG:(q + 1) * QG], src)

        for q in range(NQ):
            grams(q, last=(q == NQ - 1))
            phase_a(q)
            if q >= 1:
                phase_b(q - 1)
            if q >= 2:
                phase_c(q - 2)
        phase_c(NQ - 2)
        phase_b(NQ - 1)
        phase_c(NQ - 1)


_NC_CACHE = None


def _build_nc():
    global _NC_CACHE
    if _NC_CACHE is not None:
        return _NC_CACHE
    nc = bacc.Bacc("TRN2", target_bir_lowering=False, debug=False, num_devices=NCORES)
    xh = nc.dram_tensor("x", [P, NG, NCH, P], F8, kind="ExternalInput").ap()
    cst = nc.dram_tensor("cst", [P, 3 * P], F32, kind="ExternalInput").ap()
    y = nc.dram_tensor("y", [IPG, NG], F32, kind="ExternalOutput").ap()
    with tile.TileContext(nc) as tc:
        build_tile_kernel(tc, [y], [xh, cst])
    nc.compile()
    _NC_CACHE = nc
    return nc


def make_consts():
    idn32 = np.eye(P, dtype=np.float32)
    bd = np.kron(np.eye(IPG, dtype=np.float32), np.ones((V, V), dtype=np.float32))
    bdo = bd - np.eye(P, dtype=np.float32)
    return np.ascontiguousarray(
        np.concatenate([idn32, bdo, bd], axis=1).astype(np.float32)
    )


def shard_inputs(vf):
    """vf [V*BS, C] -> list of per-core [P, NG, NCH, P] fp8 arrays with
    xh[c, g, j, v'] = row (g*128 + v') of core k's item-major layout,
    channel j*128+c. The fp8 cast is the kernel's working precision;
    pre-transposing host-side makes the device DMA fully contiguous."""
    vf3 = np.asarray(vf, dtype=np.float32).reshape(V, BS, C)
    shards = []
    for k in range(NCORES):
        sl = vf3[:, k * BS_CORE:(k + 1) * BS_CORE, :]  # [V, 128, C]
        xk = sl.transpose(1, 0, 2).reshape(BS_CORE * V, C)  # rows: item b, view v
        xk8 = xk.astype(NP_F8)
        # [g, v', j, c] -> [c, g, j, v']
        xh = xk8.reshape(NG, P, NCH, P).transpose(3, 0, 2, 1)
        shards.append(np.ascontiguousarray(xh))
    return shards


def _run(vision_features, num_views, trace=False):
    num_views = int(np.asarray(num_views))
    assert num_views == V, f"kernel hardcoded for V=16, got {num_views}"
    vf = np.asarray(vision_features, dtype=np.float32)
    assert vf.shape == (V * BS, C), vf.shape

    nc = _build_nc()
    cst = make_consts()
    shards = shard_inputs(vf)
    in_maps = [
        {"x": shards[k], "cst": cst}
        for k in range(NCORES)
    ]
    res = run_bass_kernel_spmd(
        nc, in_maps, core_ids=list(range(NCORES)), trace=trace
    )
    outs = []
    for k in range(NCORES):
        yk = res.results[k]["y"]          # [IPG, NG], y[b, g]
        outs.append(yk.T.reshape(BS_CORE))  # index g*8+b -> local item
    full = np.concatenate(outs).astype(np.float32)  # [1024]
    return full, res


def kernel(**inputs):
    out, _ = _run(**inputs)
    return out


# revision 27
# speedup vs baseline: 1.1527x; 1.1527x over previous
"""Trainium2 Bass kernel for nn_InterViews (retrieval_knn).

Computes, per batch item b: the variance (ddof=1) of the strict-upper-
triangular entries of the cosine-similarity Gram matrix between the
item's V=16 views, negated.

Strategy (data-parallel over bs across 8 cores, 128 items/core):
  - Host: shard rows so core k holds 16 groups of 8 items (each group =
    128 rows = 8 items x 16 views), cast to fp8 e4m3, and pre-transpose
    to xh[c, g, j, v] = x[g*128+v, j*128+c] so the device DMA is pure
    streaming.
  - Device DMA: one merged-const DMA, then the 16 groups as singles
    (group 0 split in 4 slices, group 1 in 2, to cut the first-gram
    latency), all on the sync HWDGE queue; every group tile stays
    SBUF-resident so nothing is gated on buffer reuse and the 16 DMA
    engines stream the 8 MB input back-to-back (~22.5 us).  Dummy fp8
    matmuls bridge the PE's DVFS ramp (0.65 -> 1.2 -> 2.4 GHz over 3 us
    of continuous execution) while the first group is in flight.
  - Grams: per group 16 fp8 DoubleRow matmuls (256 channels each)
    accumulate G = A A^T in fp32 PSUM; 4 quads ride a 4-deep PSUM ring.
  - Postproc per CHUNK (4,4,4,2,1,1 groups — quad chunks mid-stream
    where big DVE ops amortize their fixed cost, tiny chunks at the end
    so the post-stream tail chain is one group deep):
      n2   = rowsum(G * I)  (DVE mul + grouped reduce), rec = 1/n2
      inv  = sqrt(rec) (Act), xd = I*inv (DVE, fp32r)
      ips  = BDO^T @ xd     (PE, fp32r; invT pattern), Act copy to SBUF
      tmp  = G * invT (DVE), t1 = grouped rowsum (DVE)
      wst  = tmp^2 with fused rowsum -> r2 (Act Square + accum)
      stats = (t1*inv, r2*rec) (DVE, fp32r)
      [s1',s2'] = BD^T @ stats  (PE, fp32r)
      out = ((s1')*S1SCL)^2 - s2'/238 via two activations -> y DMA.
"""

import numpy as np

try:
    import concourse.bass as bass  # noqa: F401
except ImportError:  # container installs the repo at /opt/trn_rl_repo
    import sys

    sys.path.insert(0, "/opt/trn_rl_repo")

import ml_dtypes

import concourse.bass as bass
import concourse.mybir as mybir
import concourse.tile as tile
from concourse import bacc
from concourse.bass_utils import run_bass_kernel_spmd

F32 = mybir.dt.float32
F32R = mybir.dt.float32r
F8 = mybir.dt.float8e4
NP_F8 = ml_dtypes.float8_e4m3
P = 128          # partitions / rows per group
C = 4096         # channels
V = 16           # views per item
NCORES = 8
BS = 1024        # total batch
BS_CORE = BS // NCORES   # 128 items per core
IPG = P // V             # 8 items per group
NG = BS_CORE // IPG      # 16 groups per core
NCH = C // P             # 32 channel chunks
QG = 4                   # groups per gram PSUM quad
NQ = NG // QG

# Postproc chunks: steady-state quads, then 2+1+1 so the tail chain
# after the last gram is only one group deep.
CHUNKS = [(0, 1, 2, 3), (4, 5, 6, 7), (8, 9, 10, 11), (12, 13), (14,), (15,)]

MULT = mybir.AluOpType.mult
ADD = mybir.AluOpType.add
AF = mybir.ActivationFunctionType
AXX = mybir.AxisListType.X
DR = mybir.MatmulPerfMode.DoubleRow

S1SCL = 1.0 / np.sqrt(240.0 * 238.0)

NWARM_BIG = 3     # [128, 512] fp8 warmup matmuls (~0.5 us each)
NWARM_SMALL = 5   # [128, 128] fp8 warmup matmuls


def build_tile_kernel(tc, outs, ins):
    """ins = [xh [P, NG, NCH, P] f8e4, cst [P, 3P] f32 = [I | BD-I | BD]]
    outs = [y [IPG, NG] f32]  (y[b, g] = result for local item g*8+b)
    """
    nc = tc.nc
    xh, cst = ins
    (y,) = outs

    from contextlib import ExitStack

    with ExitStack() as ctx:
        x_pool = ctx.enter_context(tc.tile_pool(name="x", bufs=NG))
        g_psum = ctx.enter_context(tc.tile_pool(name="gp", bufs=NQ, space="PSUM"))
        pp_psum = ctx.enter_context(tc.tile_pool(name="pp", bufs=2, space="PSUM"))
        j_psum = ctx.enter_context(tc.tile_pool(name="jp", bufs=1, space="PSUM"))
        ck_pool = ctx.enter_context(tc.tile_pool(name="ckp", bufs=3))
        gh_pool = ctx.enter_context(tc.tile_pool(name="ghp", bufs=3))
        sm_pool = ctx.enter_context(tc.tile_pool(name="sm", bufs=2))
        c_pool = ctx.enter_context(tc.tile_pool(name="const", bufs=1))

        jscr = j_psum.tile([32, 32], F32)

        # Warmup scratch; output borrows the first slot of the gram PSUM
        # ring (quad 3 reuses it via same-engine ordering, no extra wait).
        wsrc = c_pool.tile([P, 4, P], F8)
        nc.vector.memset(wsrc[:], 0.0)
        wdst = g_psum.tile([P, QG * P], F32, name="gps", tag="gps")

        # Consts first (one ~0.2 us DMA), then the groups, all on sync.
        cstt = c_pool.tile([P, 3 * P], F32)
        nc.sync.dma_start(cstt[:], cst[:, :])
        ident = cstt[:, 0:P]
        # BD masks feed fp32r matmuls, whose inputs must come from a
        # rounding-capable producer — rewrite them through an Act copy.
        bdrr = c_pool.tile([P, 2 * P], F32R)
        nc.scalar.copy(bdrr[:], cstt[:, P:3 * P])
        bdot = bdrr[:, 0:P]
        bdt = bdrr[:, P:2 * P]

        xtiles = [None] * NG
        for g in range(NG):
            xg = x_pool.tile([P, NCH, P], F8, tag="x")
            if g == 0:      # 4 slices: first gram can start ~3x earlier
                for s in range(4):
                    nc.sync.dma_start(xg[:, 8 * s:8 * (s + 1), :],
                                      xh[:, g, 8 * s:8 * (s + 1), :])
            elif g == 1:    # 2 slices
                for s in range(2):
                    nc.sync.dma_start(xg[:, 16 * s:16 * (s + 1), :],
                                      xh[:, g, 16 * s:16 * (s + 1), :])
            else:
                nc.sync.dma_start(xg[:, :, :], xh[:, g, :, :])
            xtiles[g] = xg

        # PE warmup: runs while group 0 is still in flight.
        for i in range(NWARM_BIG):
            nc.tensor.matmul(wdst[:], wsrc[:, 0, :], wsrc[:],
                             skip_group_check=True)
        for i in range(NWARM_SMALL):
            nc.tensor.matmul(wdst[:, 0:P], wsrc[:, 0, :], wsrc[:, 0, :],
                             skip_group_check=True)
        # Absorb the const-DMA wait into PE's observed clock (TRN2
        # Matmult carries at most one semaphore wait).
        nc.tensor.matmul(jscr[:], cstt[0:32, 0:32], cstt[0:32, 0:32],
                         skip_group_check=True)

        stage = c_pool.tile([P, NG], F32)

        gtiles = [None] * NQ
        cstate = {}           # chunk -> dict(xd=, stats=)

        def gram_group(g):
            q, gl = divmod(g, QG)
            if gl == 0:
                gtiles[q] = g_psum.tile([P, QG * P], F32, name="gps", tag="gps")
            gps = gtiles[q]
            xg = xtiles[g]
            for j in range(NCH // 2):
                nc.tensor.matmul(
                    gps[:, gl * P:(gl + 1) * P],
                    xg[:, 2 * j:2 * j + 2, :],
                    xg[:, 2 * j:2 * j + 2, :],
                    start=(j == 0),
                    stop=(j == NCH // 2 - 1),
                    perf_mode=DR,
                    skip_group_check=True,
                )

        def grams(q):
            for gl in range(QG):
                gram_group(q * QG + gl)

        def post_a(ci):
            """Chunk postproc part A, issued once the chunk's grams are
            done: n2 = diag(G) via mask-mul + grouped rowsum, rec = 1/n2
            (DVE), inv = sqrt(rec) (Act), xd = I*inv (DVE, fp32r)."""
            ch = CHUNKS[ci]
            nl = len(ch)
            q = ch[0] // QG
            gsl = gtiles[q][:, (ch[0] % QG) * P:((ch[0] % QG) + nl) * P]
            st = cstate.setdefault(ci, {})
            idb = ident.unsqueeze(1).broadcast_to([P, nl, P])
            scrc = gh_pool.tile([P, nl * P], F32, tag="scr", name="scrc")
            nc.vector.tensor_mul(
                scrc[:].rearrange("p (i q) -> p i q", i=nl),
                gsl.rearrange("p (i q) -> p i q", i=nl), idb,
            )
            n2c = ck_pool.tile([P, nl], F32, tag="n2", name="n2c")
            nc.vector.reduce_sum(
                n2c[:], scrc[:].rearrange("p (i q) -> p i q", i=nl), axis=AXX
            )
            recc = ck_pool.tile([P, nl], F32, tag="rec", name="recc")
            nc.vector.reciprocal(recc[:], n2c[:])
            invc = ck_pool.tile([P, nl], F32, tag="invc", name="invc")
            nc.scalar.activation(invc[:], recc[:], AF.Sqrt)
            st["rec"], st["inv"] = recc, invc
            xdc = ck_pool.tile([P, nl * P], F32R, tag="xd", name="xdc")
            invb = invc[:].unsqueeze(2).broadcast_to([P, nl, P])
            nc.vector.tensor_mul(
                xdc[:].rearrange("p (i q) -> p i q", i=nl), idb, invb
            )
            st["xd"] = xdc

        def ips(ci):
            """invT = BDO^T @ xd (PE fp32r) -> Act copy -> tmp = G*invT,
            t1 = grouped rowsum (DVE); wst = tmp^2 with fused rowsum ->
            r2 (Act Square+accum); stats = (t1*inv, r2*rec) (DVE)."""
            ch = CHUNKS[ci]
            nl = len(ch)
            w = nl * P
            st = cstate[ci]
            q = ch[0] // QG
            gsl = gtiles[q][:, (ch[0] % QG) * P:((ch[0] % QG) + nl) * P]
            ipst = pp_psum.tile([P, QG * P], F32, tag="ips")
            nc.tensor.matmul(ipst[:, 0:w], bdot, st["xd"][:],
                             skip_group_check=True)
            invT = gh_pool.tile([P, nl * P], F32, tag="invT", name="invTc")
            nc.scalar.copy(invT[:], ipst[:, 0:w])
            tmpc = gh_pool.tile([P, nl * P], F32, tag="tmp", name="tmpc")
            nc.vector.tensor_mul(tmpc[:], gsl, invT[:])
            t1c = ck_pool.tile([P, nl], F32, tag="t1", name="t1c")
            nc.vector.reduce_sum(
                t1c[:], tmpc[:].rearrange("p (i q) -> p i q", i=nl), axis=AXX
            )
            stats = ck_pool.tile([P, 2 * nl], F32R, tag="stats", name="statc")
            st["stats"] = stats
            r2c = ck_pool.tile([P, nl], F32, tag="r2", name="r2c")
            for pos in range(nl):
                wsg = gh_pool.tile([P, P], F32, tag="wst")
                nc.scalar.activation(
                    wsg[:], tmpc[:, pos * P:(pos + 1) * P], AF.Square,
                    accum_out=r2c[:, pos:pos + 1],
                )
            nc.vector.tensor_mul(stats[:, 0:2 * nl:2], t1c[:], st["inv"][:])
            nc.vector.tensor_mul(stats[:, 1:2 * nl:2], r2c[:], st["rec"][:])

        def fin(ci):
            """[s1',s2'] = BD^T @ stats (fp32r) -> y slice."""
            ch = CHUNKS[ci]
            st = cstate[ci]
            w = 2 * len(ch)
            sps = j_psum.tile([P, 2 * QG], F32, tag="sps")
            nc.tensor.matmul(sps[:, 0:w], bdt, st["stats"][:],
                             skip_group_check=True)
            # out = (s1*S1SCL)^2 - s2/238  (= -var)
            qv = sm_pool.tile([P, QG], F32, tag="qv")
            wv = sm_pool.tile([P, QG], F32, tag="wv")
            nl = len(ch)
            nc.scalar.activation(qv[:, 0:nl], sps[:, 0:w:2], AF.Square,
                                 scale=S1SCL)
            nc.scalar.mul(wv[:, 0:nl], sps[:, 1:w:2], -1.0 / 238.0)
            c0 = ch[0]
            nc.vector.tensor_add(stage[:, c0:c0 + nl], qv[:, 0:nl], wv[:, 0:nl])
            src = stage[:].rearrange("(b r) g -> b r g", r=V)[:, 0, c0:c0 + nl]
            nc.sync.dma_start(y[:, c0:c0 + nl], src)

        grams(0)
        post_a(0)
        grams(1)
        post_a(1)
        ips(0)
        grams(2)
        post_a(2)
        ips(1)
        fin(0)
        gram_group(12)
        gram_group(13)
        post_a(3)
        ips(2)
        gram_group(14)
        post_a(4)
        fin(1)
        gram_group(15)
        post_a(5)
        ips(3)
        ips(4)
        fin(2)
        ips(5)
        fin(3)
        fin(4)
        fin(5)


_NC_CACHE = None


def _build_nc():
    global _NC_CACHE
    if _NC_CACHE is not None:
        return _NC_CACHE
    nc = bacc.Bacc("TRN2", target_bir_lowering=False, debug=False, num_devices=NCORES)
    xh = nc.dram_tensor("x", [P, NG, NCH, P], F8, kind="ExternalInput").ap()
    cst = nc.dram_tensor("cst", [P, 3 * P], F32, kind="ExternalInput").ap()
    y = nc.dram_tensor("y", [IPG, NG], F32, kind="ExternalOutput").ap()
    with tile.TileContext(nc) as tc:
        build_tile_kernel(tc, [y], [xh, cst])
    nc.compile()
    _NC_CACHE = nc
    return nc


def make_consts():
    idn32 = np.eye(P, dtype=np.float32)
    bd = np.kron(np.eye(IPG, dtype=np.float32), np.ones((V, V), dtype=np.float32))
    bdo = bd - np.eye(P, dtype=np.float32)
    return np.ascontiguousarray(
        np.concatenate([idn32, bdo, bd], axis=1).astype(np.float32)
    )


def shard_inputs(vf):
    """vf [V*BS, C] -> list of per-core [P, NG, NCH, P] fp8 arrays with
    xh[c, g, j, v'] = row (g*128 + v') of core k's item-major layout,
    channel j*128+c. The fp8 cast is the kernel's working precision;
    pre-transposing host-side makes the device DMA fully contiguous."""
    vf3 = np.asarray(vf, dtype=np.float32).reshape(V, BS, C)
    shards = []
    for k in range(NCORES):
        sl = vf3[:, k * BS_CORE:(k + 1) * BS_CORE, :]  # [V, 128, C]
        xk = sl.transpose(1, 0, 2).reshape(BS_CORE * V, C)  # rows: item b, view v
        xk8 = xk.astype(NP_F8)
        # [g, v', j, c] -> [c, g, j, v']
        xh = xk8.reshape(NG, P, NCH, P).transpose(3, 0, 2, 1)
        shards.append(np.ascontiguousarray(xh))
    return shards


def _run(vision_features, num_views, trace=False):
    num_views = int(np.asarray(num_views))
    assert num_views == V, f"kernel hardcoded for V=16, got {num_views}"
    vf = np.asarray(vision_features, dtype=np.float32)
    assert vf.shape == (V * BS, C), vf.shape

    nc = _build_nc()
    cst = make_consts()
    shards = shard_inputs(vf)
    in_maps = [
        {"x": shards[k], "cst": cst}
        for k in range(NCORES)
    ]
    res = run_bass_kernel_spmd(
        nc, in_maps, core_ids=list(range(NCORES)), trace=trace
    )
    outs = []
    for k in range(NCORES):
        yk = res.results[k]["y"]          # [IPG, NG], y[b, g]
        outs.append(yk.T.reshape(BS_CORE))  # index g*8+b -> local item
    full = np.concatenate(outs).astype(np.float32)  # [1024]
    return full, res


def kernel(**inputs):
    out, _ = _run(**inputs)
    return out
